# revision 1
# baseline (speedup 1.0000x reference)
"""Trainium2 Bass kernel for nn_BoundaryLoss (3D boundary/dice loss).

Math: for pred/target volumes [2,1,192,192,192] f32,
  b(x) = sqrt(gx^2+gy^2+gz^2+1e-5) with central differences (zero pad),
  loss = 1 - (2*sum(pb*tb)+s)/(sum(pb)+sum(tb)+s).

Sharding: 8 cores = 2 batches x 4 depth-quarters (48 slices each, 1-slice
halo).  Each core computes 3 partial sums; host combines.

Per-core layout: a tensor shard is [H=192 rows, 50 slices x 196 cols] fp16
(W padded 192->196 with zeros at cols {0,1,194,195}; data col j = w+2).
H is split into chunk A (partitions 0..127, valid h 0..126) and chunk B
(rows 120..191 on 72 partitions, valid h 127..191).  With (d,w) flattened
on the free axis:
  gx (depth diff)  = flat shift by +-196  -> fused sq-diff on DVE
  gz (width diff)  = flat shift by +-1    -> fused sq-diff on DVE
  gy (height diff) = partition shift      -> PE matmul with +-1 shift matrix
Then V = (gx2+gz2)+gy2 (two DVE TT adds; gy2 via ACT Square from PSUM),
pb = ACT Sqrt(V + eps-bias) with per-partition accum (sum pb), and
sum(pb*tb) via GPSIMD scalar_tensor_tensor with fused accum.  All
accumulator slots are f32; host sums in f64.

Container quirks worked around here: walrus accepts at most ONE semaphore
wait per instruction (excess waits are split onto EventSemaphore
instructions at the serialized-BIR level via a to_json_bytes patch), and
raw-ISA instructions (custom DVE ops, tensor_tensor_reduce) are rejected
("ISA wrong length"), so only standard BIR opcodes are used.
"""

import sys

sys.path.insert(0, "/opt/trn_rl_repo")

import numpy as np

# ---------------- problem constants (hardcoded per contract) ----------------
BATCH = 2
DVOL = 192           # full depth
H = 192
W = 192
NCORES = 8
NQ = 4               # depth quarters per batch
DL = DVOL // NQ      # 48 local slices per core
S = DL + 2           # 50 slices incl halo
WP = W + 4           # 196 padded row
FREE = S * WP        # 9800
OUTC = DL * WP       # 9408 output cols per chunk
SBC = 1568           # sub-block cols (8 slices x 196)
NSB = OUTC // SBC    # 6
SLICES_PER_SB = SBC // WP  # 8
EPS = 1e-5
B0 = 120             # chunk B first H row
PA, PB_ = 128, 72    # partitions per chunk
# valid partition ranges [lo, hi) for accumulation
VA = (0, 127)        # chunk A covers h 0..126
VB = (7, 72)         # chunk B covers h 127..191

_NC_CACHE = {}

# this container's walrus rejects instructions carrying more than a couple
# of semaphore waits ("Too many sync wait commands" on the Tile tail drain).
# Split excess waits onto same-engine Drain instructions inserted just
# before the offender, at the serialized-BIR level (single choke point for
# both the PJRT/axon path and compile_bass_kernel).
_WAIT_CAP = 1


def _split_multiwait_json(bs: bytes) -> bytes:
    import json

    m = json.loads(bs)
    changed = False
    for fn in m.get("functions", []):
        for blk in fn.get("blocks", []):
            insts = blk.get("instructions")
            if not insts:
                continue
            out = []
            for ins in insts:
                si = ins.get("sync_info") or {}
                ow = si.get("on_wait") or []
                if len(ow) > _WAIT_CAP:
                    chunks = [
                        ow[i : i + _WAIT_CAP] for i in range(0, len(ow), _WAIT_CAP)
                    ]
                    for ci, ch in enumerate(chunks[:-1]):
                        out.append(
                            {
                                "debug": ins.get("debug", 0),
                                "engine": ins["engine"],
                                "ins": [],
                                "outs": [],
                                "is_reset_sema": False,
                                "name": f"{ins['name']}__w{ci}",
                                "opcode": "EventSemaphore",
                                "sync_info": {"on_update": [], "on_wait": ch},
                            }
                        )
                    si["on_wait"] = chunks[-1]
                    ins["sync_info"] = si
                    changed = True
                out.append(ins)
            blk["instructions"] = out
    if not changed:
        return bs
    return json.dumps(m).encode()


def _install_json_patch():
    import concourse.bass as bass

    if getattr(bass.Bass, "_bl_json_patched", False):
        return
    orig = bass.Bass.to_json_bytes

    def to_json_bytes(self, *a, **k):
        return _split_multiwait_json(orig(self, *a, **k))

    bass.Bass.to_json_bytes = to_json_bytes
    bass.Bass._bl_json_patched = True


# ---------------- custom DVE op: out = (in0 - in1)^2 ----------------
def _register_sqdiff():
    import concourse.dve_ops as dve_ops
    from concourse.dve_spec import Spec, Src0, Src1, lower, sq
    from concourse.dve_uop import DveOpSpec

    name = "SQDIFF_BL"
    for op in dve_ops.OPS:
        if op.name == name:
            return op
    spec = Spec(
        body=sq(Src0 - Src1),
        reference=lambda in0, in1, s0, s1, imm2: (
            in0.astype(np.float32) - in1.astype(np.float32)
        )
        ** 2,
    )
    shas = {}
    for ver in ("v3", "v4"):
        s = DveOpSpec(name=name, opcode=1, uops=lower(spec, ver=ver), rd1_en=True)
        shas[ver] = s.sha(ver)
    op = dve_ops.DveOp(name, spec, subdim=False, uops_sha=shas)
    row = max(dve_ops._SUB_OPCODE_FOR_NAME.values()) + 1
    assert row < 0x20
    dve_ops.OPS.append(op)
    dve_ops.CUSTOM_DVE_SPECS[name] = spec
    dve_ops._SUB_OPCODE_FOR_NAME[name] = row
    return op


# ---------------- device program ----------------
def build_nc(repeats=1, variant="psum_acc", sbc=SBC, work_bufs=3, pb_bufs=2, dma_pieces=4, act_gx2=3):
    from contextlib import ExitStack

    import concourse.bass as bass
    import concourse.mybir as mybir
    from concourse import tile

    _install_json_patch()

    f16 = mybir.dt.float16
    f32 = mybir.dt.float32
    ADD = mybir.AluOpType.add
    MULT = mybir.AluOpType.mult
    SQUARE = mybir.ActivationFunctionType.Square
    SQRT = mybir.ActivationFunctionType.Sqrt
    AXX = mybir.AxisListType.X

    nc = bass.Bass("TRN2", target_bir_lowering=False, debug=False)

    xp = nc.dram_tensor("xp", [H, FREE], f16, kind="ExternalInput")
    xt = nc.dram_tensor("xt", [H, FREE], f16, kind="ExternalInput")
    da = nc.dram_tensor("da", [PA, PA], f16, kind="ExternalInput")
    db = nc.dram_tensor("db", [PB_, PB_], f16, kind="ExternalInput")
    ia = nc.dram_tensor("ia", [PA, PA], f16, kind="ExternalInput")
    ib = nc.dram_tensor("ib", [PB_, PB_], f16, kind="ExternalInput")
    out = nc.dram_tensor("o", [128, 8], f32, kind="ExternalOutput")

    # matmul windows within one sub-block (each inside one PSUM bank)
    nsb = OUTC // sbc
    assert nsb * sbc == OUTC
    spsb = sbc // WP  # slices per sub-block
    MMW = []
    w0 = 0
    while w0 < sbc:
        MMW.append((w0, min(512, sbc - w0)))
        w0 += 512
    psum_banks = -(-sbc * 4 // 2048)  # banks per psum tile
    psum_bufs = max(2, 8 // psum_banks)

    with tile.TileContext(nc) as tc, ExitStack() as ctx:
        const = ctx.enter_context(tc.tile_pool(name="const", bufs=1))
        xpool = ctx.enter_context(tc.tile_pool(name="x", bufs=1))
        work = ctx.enter_context(tc.tile_pool(name="work", bufs=work_bufs))
        pbp = ctx.enter_context(tc.tile_pool(name="pb", bufs=pb_bufs))
        accp = ctx.enter_context(tc.tile_pool(name="acc", bufs=1))
        psum = ctx.enter_context(
            tc.tile_pool(name="psum", bufs=psum_bufs, space="PSUM")
        )

        da_t = const.tile([PA, PA], f16, tag="da")
        nc.sync.dma_start(da_t[:], da[:, :])
        eps_t = const.tile([128, 1], f32, tag="eps")
        nc.vector.memset(eps_t[:], EPS)
        db_t = const.tile([PB_, PB_], f16, tag="db")
        nc.sync.dma_start(db_t[:], db[:, :])
        ia_t = const.tile([PA, PA], f16, tag="ia")
        nc.sync.dma_start(ia_t[:], ia[:, :])
        ib_t = const.tile([PB_, PB_], f16, tag="ib")
        nc.sync.dma_start(ib_t[:], ib[:, :])

        X = {}
        for tname, dram in (("p", xp), ("t", xt)):
            for ch, pc, r0 in (("A", PA, 0), ("B", PB_, B0)):
                t_ = xpool.tile([pc, FREE], f16, tag=f"x{tname}{ch}")
                # split the load across DMA queues for bandwidth + overlap
                step = -(-FREE // dma_pieces)
                for p0 in range(0, FREE, step):
                    p1 = min(FREE, p0 + step)
                    nc.sync.dma_start(
                        t_[:, p0:p1], dram[r0 : r0 + pc, p0:p1]
                    )
                X[tname, ch] = t_

        # accumulator slot tiles: per (quantity, chunk), one f32 col per sub-block
        SA = {}
        nslots = nsb * repeats
        for q in ("sp", "st", "pt"):
            for ch in ("A", "B"):
                SA[q, ch] = accp.tile(
                    [128, nslots], f32, tag=f"sa_{q}_{ch}", name=f"sa_{q}_{ch}"
                )

        gx2_moved = [0]  # how many gx^2 squares sent to ACT so far
        for rep in range(repeats):
            for ch, pc, dmat, imat, (vlo, vhi) in (
                ("A", PA, da_t, ia_t, VA),
                ("B", PB_, db_t, ib_t, VB),
            ):
                for sb in range(nsb):
                    c0 = sb * sbc
                    PBt = {}
                    for tname in ("p", "t"):
                        x = X[tname, ch]
                        # gx^2: depth central diff, flat shift +-196
                        gx = work.tile([pc, sbc], f16, tag="gx")
                        nc.vector.tensor_sub(
                            gx[:],
                            x[:, c0 + 392 : c0 + 392 + sbc],
                            x[:, c0 : c0 + sbc],
                        )
                        gx2 = work.tile([pc, sbc], f16, tag="gx2")
                        if gx2_moved[0] < act_gx2 * repeats:
                            gx2_moved[0] += 1
                            nc.scalar.activation(gx2[:], gx[:], SQUARE)
                        else:
                            nc.vector.tensor_mul(gx2[:], gx[:], gx[:])
                        # gz^2: width central diff, flat shift +-1 (center +196)
                        gz = work.tile([pc, sbc], f16, tag="gz")
                        nc.gpsimd.tensor_sub(
                            gz[:],
                            x[:, c0 + 197 : c0 + 197 + sbc],
                            x[:, c0 + 195 : c0 + 195 + sbc],
                        )
                        gz2 = work.tile([pc, sbc], f16, tag="gz2")
                        nc.vector.tensor_mul(gz2[:], gz[:], gz[:])
                        # gy via PE shift-matmul into PSUM, then ACT square
                        ps = psum.tile([pc, sbc], f32, tag="ps")
                        for w0, wn in MMW:
                            nc.tensor.matmul(
                                ps[:, w0 : w0 + wn],
                                dmat[:],
                                x[:, 196 + c0 + w0 : 196 + c0 + w0 + wn],
                                start=True,
                                stop=True,
                            )
                        q_ = work.tile([pc, sbc], f16, tag="q")
                        nc.scalar.activation(q_[:], ps[:], SQUARE)
                        if variant == "psum_acc":
                            # v = gx2+gz2+gy2 accumulated into the gy psum
                            # via identity matmuls on PE (no DVE adds)
                            for w0, wn in MMW:
                                nc.tensor.matmul(
                                    ps[:, w0 : w0 + wn],
                                    imat[:],
                                    gx2[:, w0 : w0 + wn],
                                    start=True,
                                    stop=False,
                                )
                                nc.tensor.matmul(
                                    ps[:, w0 : w0 + wn],
                                    imat[:],
                                    gz2[:, w0 : w0 + wn],
                                    start=False,
                                    stop=False,
                                )
                                nc.tensor.matmul(
                                    ps[:, w0 : w0 + wn],
                                    imat[:],
                                    q_[:, w0 : w0 + wn],
                                    start=False,
                                    stop=True,
                                )
                            vsrc = ps
                        else:
                            # DVE adds: v = (gx2 + gz2) + gy2
                            v0 = work.tile([pc, sbc], f16, tag="v0")
                            nc.vector.tensor_add(v0[:], gx2[:], gz2[:])
                            v1 = work.tile([pc, sbc], f16, tag="v1")
                            nc.vector.tensor_add(v1[:], v0[:], q_[:])
                            vsrc = v1
                        # pb = sqrt(v + eps) on data cols, accum = row sums
                        pb = pbp.tile([pc, spsb * W], f16, tag=f"pb{tname}")
                        v3 = vsrc[:].rearrange("p (s w) -> p s w", s=spsb)
                        pb3 = pb[:].rearrange("p (s w) -> p s w", s=spsb)
                        qn = "sp" if tname == "p" else "st"
                        nc.scalar.activation(
                            pb3[:, :, :],
                            v3[:, :, 2 : 2 + W],
                            SQRT,
                            bias=eps_t[0:pc, :],
                            accum_out=SA[qn, ch][
                                0:pc, rep * nsb + sb : rep * nsb + sb + 1
                            ],
                        )
                        PBt[tname] = pb
                    # sum(pb*tb) for this sub-block: (pb*1.0)*tb with fused accum
                    prod = work.tile([pc, spsb * W], f16, tag="prod")
                    nc.vector.scalar_tensor_tensor(
                        prod[:, :],
                        PBt["p"][:, :],
                        1.0,
                        PBt["t"][:, :],
                        op0=MULT,
                        op1=MULT,
                        accum_out=SA["pt", ch][
                            0:pc, rep * nsb + sb : rep * nsb + sb + 1
                        ],
                    )

        # reduce slot columns and write partials to DRAM
        colmap = [
            ("sp", "A"), ("sp", "B"),
            ("st", "A"), ("st", "B"),
            ("pt", "A"), ("pt", "B"),
        ]
        for col, (q, ch) in enumerate(colmap):
            vlo, vhi = VA if ch == "A" else VB
            pc = PA if ch == "A" else PB_
            r = accp.tile([128, 1], f32, tag=f"red{col}")
            nc.vector.tensor_reduce(
                r[0:pc, :], SA[q, ch][0:pc, 0:nslots], AXX, ADD
            )
            nc.sync.dma_start(out[vlo:vhi, col : col + 1], r[vlo:vhi, :])

    return nc


def get_nc():
    if "nc" not in _NC_CACHE:
        _NC_CACHE["nc"] = build_nc()
    return _NC_CACHE["nc"]


# ---------------- host-side sharding ----------------
def _dmat(k):
    d = np.zeros((k, k), np.float16)
    for m in range(k):
        if m + 1 < k:
            d[m + 1, m] = 1.0
        if m - 1 >= 0:
            d[m - 1, m] = -1.0
    return d


DA_NP = _dmat(PA)
DB_NP = _dmat(PB_)
IA_NP = np.eye(PA, dtype=np.float16)
IB_NP = np.eye(PB_, dtype=np.float16)


def _shard(vol, q):
    """vol [192,192,192] f32 -> [H, FREE] fp16 padded shard for quarter q."""
    sh = np.zeros((S, H, WP), np.float16)
    d0 = DL * q - 1
    lo, hi = max(d0, 0), min(d0 + S, DVOL)
    sh[lo - d0 : hi - d0, :, 2 : 2 + W] = vol[lo:hi].astype(np.float16)
    # -> [H, S, WP] -> [H, FREE]
    return np.ascontiguousarray(sh.transpose(1, 0, 2)).reshape(H, FREE)


def make_in_maps(pred, target):
    pred = np.asarray(pred, dtype=np.float32).reshape(BATCH, DVOL, H, W)
    target = np.asarray(target, dtype=np.float32).reshape(BATCH, DVOL, H, W)
    maps = []
    for c in range(NCORES):
        b, q = divmod(c, NQ)
        maps.append(
            {
                "xp": _shard(pred[b], q),
                "xt": _shard(target[b], q),
                "da": DA_NP,
                "db": DB_NP,
                "ia": IA_NP,
                "ib": IB_NP,
            }
        )
    return maps


def combine(results):
    sp = st = pt = 0.0
    a0, a1 = VA
    b0, b1 = VB
    for r in results:
        o = np.asarray(r["o"], dtype=np.float64)
        sp += o[a0:a1, 0].sum() + o[b0:b1, 1].sum()
        st += o[a0:a1, 2].sum() + o[b0:b1, 3].sum()
        pt += o[a0:a1, 4].sum() + o[b0:b1, 5].sum()
    dice = (2.0 * pt + EPS) / (sp + st + EPS)
    return np.float32(1.0 - dice)


def run_on_device(in_maps, **kwargs):
    from concourse.bass_utils import run_bass_kernel_spmd

    nc = get_nc()
    return run_bass_kernel_spmd(nc, in_maps, core_ids=list(range(NCORES)), **kwargs)


def kernel(pred, target):
    in_maps = make_in_maps(pred, target)
    res = run_on_device(in_maps)
    return combine(res.results)


if __name__ == "__main__":
    rng = np.random.default_rng(0)
    p = rng.random((2, 1, 192, 192, 192), np.float32)
    t = rng.random((2, 1, 192, 192, 192), np.float32)
    print(kernel(p, t))



# revision 2
# speedup vs baseline: 1.0276x; 1.0276x over previous
"""Kernel v3: 3-tile row packing — [pred h0:128], [tgt h0:128],
[pred h128:192 | tgt h128:192] — so every op uses all 128 partitions
(25% fewer columns than the 2-chunk layout) and all rows are valid.

Height-diff gy via per-tile shift matmuls; the 2-row cross-tile
boundaries (h=127/128 of each tensor) are stitched with extra matmuls
from the neighboring tile accumulated into the same PSUM. T3's
pb(pred)*tb(tgt) product needs lane alignment: DMA-realign rows 64:128
onto 0:64 then TSP. Dice needs only Σpb+Σtb so per-tile sqrt accums sum
host-side.
"""

import sys

sys.path.insert(0, "/opt/trn_rl_repo")

import numpy as np

BATCH = 2
DVOL = 192
H = 192
W = 192
NCORES = 8
NQ = 4
DL = DVOL // NQ      # 48
S = DL + 2           # 50
WP = W + 4           # 196
FREE = S * WP        # 9800
EPS = 1e-5
HB = 64              # T3 block height

_NC_CACHE = {}
_WAIT_CAP = 1


def _split_multiwait_json(bs: bytes) -> bytes:
    import json

    m = json.loads(bs)
    changed = False
    for fn in m.get("functions", []):
        for blk in fn.get("blocks", []):
            insts = blk.get("instructions")
            if not insts:
                continue
            out = []
            for ins in insts:
                si = ins.get("sync_info") or {}
                ow = si.get("on_wait") or []
                if len(ow) > _WAIT_CAP:
                    chunks = [
                        ow[i : i + _WAIT_CAP] for i in range(0, len(ow), _WAIT_CAP)
                    ]
                    for ci, ch in enumerate(chunks[:-1]):
                        out.append(
                            {
                                "debug": ins.get("debug", 0),
                                "engine": ins["engine"],
                                "ins": [],
                                "outs": [],
                                "is_reset_sema": False,
                                "name": f"{ins['name']}__w{ci}",
                                "opcode": "EventSemaphore",
                                "sync_info": {"on_update": [], "on_wait": ch},
                            }
                        )
                    si["on_wait"] = chunks[-1]
                    ins["sync_info"] = si
                    changed = True
                out.append(ins)
            blk["instructions"] = out
    if not changed:
        return bs
    return json.dumps(m).encode()


def _install_json_patch():
    import concourse.bass as bass

    if getattr(bass.Bass, "_bl_json_patched", False):
        return
    orig = bass.Bass.to_json_bytes

    def to_json_bytes(self, *a, **k):
        return _split_multiwait_json(orig(self, *a, **k))

    bass.Bass.to_json_bytes = to_json_bytes
    bass.Bass._bl_json_patched = True


def build_nc(
    repeats=1,
    sbsl=4,            # slices per sub-block
    vmode="psum_acc",  # "psum_acc" | "mixed"
    n_subpool=17,      # of 36 units: both subs on Pool (Bresenham-spread)
    n_gx2act=0,
    n_gz2act=0,
    lag=2,
    work_bufs=5,
    pb_bufs=3,
    dma_pieces=6,
):
    from contextlib import ExitStack

    import concourse.bass as bass
    import concourse.mybir as mybir
    from concourse import tile

    _install_json_patch()

    f16 = mybir.dt.float16
    f32 = mybir.dt.float32
    MULT = mybir.AluOpType.mult
    ADD = mybir.AluOpType.add
    SQUARE = mybir.ActivationFunctionType.Square
    SQRT = mybir.ActivationFunctionType.Sqrt
    AXX = mybir.AxisListType.X

    sbc = sbsl * WP
    OUTC = DL * WP
    nsb = OUTC // sbc
    assert nsb * sbc == OUTC
    MMW = []
    w0 = 0
    while w0 < sbc:
        MMW.append((w0, min(512, sbc - w0)))
        w0 += 512
    psum_banks = -(-sbc * 4 // 2048)   # per ps tile (unfused, fp32)
    if vmode == "mixed":
        psum_bufs = max(1, 8 // (2 * psum_banks))
    else:
        psum_bufs = min(4, max(2, 8 // psum_banks))

    nc = bass.Bass("TRN2", target_bir_lowering=False, debug=False)

    xp = nc.dram_tensor("xp", [H, FREE], f16, kind="ExternalInput")
    xt = nc.dram_tensor("xt", [H, FREE], f16, kind="ExternalInput")
    mats = nc.dram_tensor("mats", [128, 7 * 128], f16, kind="ExternalInput")
    out = nc.dram_tensor("o", [128, 5], f32, kind="ExternalOutput")

    with tile.TileContext(nc) as tc, ExitStack() as ctx:
        const = ctx.enter_context(tc.tile_pool(name="const", bufs=1))
        xpool = ctx.enter_context(tc.tile_pool(name="x", bufs=1))
        work = ctx.enter_context(tc.tile_pool(name="work", bufs=work_bufs))
        pbp = ctx.enter_context(tc.tile_pool(name="pb", bufs=pb_bufs))
        accp = ctx.enter_context(tc.tile_pool(name="acc", bufs=1))
        psum = ctx.enter_context(
            tc.tile_pool(name="psum", bufs=psum_bufs, space="PSUM")
        )

        mt = const.tile([128, 7 * 128], f16, tag="mats")
        nc.sync.dma_start(mt[:], mats[:, :])
        M1 = mt[:, 0:128]          # within-tile shift for T1/T2
        M3 = mt[:, 128:256]        # block-diag shift for T3
        B1 = mt[:, 256:384]        # T3 -> T1 psum (+x[128] to row 127)
        B2 = mt[:, 384:512]        # T3 -> T2 psum
        B3 = mt[:, 512:640]        # T1 -> T3 psum (-x[127] to row 0)
        B4 = mt[:, 640:768]        # T2 -> T3 psum (-X[127] to row 64)
        I_ = mt[:, 768:896]        # identity
        eps_t = const.tile([128, 1], f32, tag="eps")
        nc.vector.memset(eps_t[:], EPS)

        # tiles: T1 = pred rows 0:128, T2 = tgt rows 0:128,
        #        T3 = pred rows 128:192 on p0:64, tgt rows 128:192 on p64:128
        T = {}
        for k in (1, 2, 3):
            T[k] = xpool.tile([128, FREE], f16, tag=f"t{k}", name=f"t{k}")
        first = sbc + 392
        csteps = [(0, first)]
        cstep = -(-(FREE - first) // dma_pieces)
        c = first
        while c < FREE:
            csteps.append((c, min(FREE, c + cstep)))
            c += cstep
        for c0_, c1_ in csteps:
            nc.sync.dma_start(T[1][:, c0_:c1_], xp[0:128, c0_:c1_])
            nc.sync.dma_start(T[2][:, c0_:c1_], xt[0:128, c0_:c1_])
            nc.sync.dma_start(T[3][0:HB, c0_:c1_], xp[128:192, c0_:c1_])
            nc.sync.dma_start(T[3][HB:128, c0_:c1_], xt[128:192, c0_:c1_])

        # accum slots
        SA = {}
        nslots = nsb * repeats
        for q in ("s1", "s2", "s3", "pa", "pb"):
            SA[q] = accp.tile([128, nslots], f32, tag=f"sa_{q}", name=f"sa_{q}")

        spsb = sbsl
        ucount = [0]
        units = []
        for rep in range(repeats):
            for sb in range(nsb):
                for tk in (3, 1, 2):
                    units.append((rep, tk, sb))

        # per-(tile,sb) shift matmul plans: (main_mat, [(bnd_mat, src_tile)...])
        PBTILE = {}  # (tk, sb) -> pb tile (for products)

        def emit_p1(unit):
            rep, tk, sb = unit
            u = ucount[0]
            ucount[0] += 1
            x = T[tk]
            c0 = sb * sbc
            gx = work.tile([128, sbc], f16, tag="gx")
            on_pool = ((u % 36) * n_subpool) % 36 < n_subpool
            eng = nc.gpsimd if on_pool else nc.vector
            eng.tensor_sub(
                gx[:], x[:, c0 + 392 : c0 + 392 + sbc], x[:, c0 : c0 + sbc]
            )
            gx2 = work.tile([128, sbc], f16, tag="gx2")
            if ((u % 36) * n_gx2act) % 36 < n_gx2act:
                nc.scalar.activation(gx2[:], gx[:], SQUARE)
            else:
                nc.vector.tensor_mul(gx2[:], gx[:], gx[:])
            gz = work.tile([128, sbc], f16, tag="gz")
            eng2 = nc.gpsimd if on_pool else nc.vector
            eng2.tensor_sub(
                gz[:],
                x[:, c0 + 197 : c0 + 197 + sbc],
                x[:, c0 + 195 : c0 + 195 + sbc],
            )
            gz2 = work.tile([128, sbc], f16, tag="gz2")
            if ((u % 36) * n_gz2act) % 36 < n_gz2act:
                nc.scalar.activation(gz2[:], gz[:], SQUARE)
            else:
                nc.vector.tensor_mul(gz2[:], gz[:], gz[:])
            # gy into psum: main shift matmul + boundary stitches
            ps = psum.tile([128, sbc], f32, tag="ps")
            if tk == 1:
                plan = [(M1, T[1], True, False), (B1, T[3], False, True)]
            elif tk == 2:
                plan = [(M1, T[2], True, False), (B2, T[3], False, True)]
            else:
                plan = [
                    (M3, T[3], True, False),
                    (B3, T[1], False, False),
                    (B4, T[2], False, True),
                ]
            for w0, wn in MMW:
                for mat, src, st, sp in plan:
                    nc.tensor.matmul(
                        ps[:, w0 : w0 + wn],
                        mat,
                        src[:, 196 + c0 + w0 : 196 + c0 + w0 + wn],
                        start=st,
                        stop=sp,
                    )
            return (u, unit, c0, gx2, gz2, ps)

        def emit_p2(st_):
            u, unit, c0, gx2, gz2, ps = st_
            rep, tk, sb = unit
            if vmode == "psum_acc":
                q_ = work.tile([128, sbc], f16, tag="q")
                nc.scalar.activation(q_[:], ps[:], SQUARE)
                for w0, wn in MMW:
                    nc.tensor.matmul(
                        ps[:, w0 : w0 + wn], I_, gx2[:, w0 : w0 + wn],
                        start=True, stop=False,
                    )
                    nc.tensor.matmul(
                        ps[:, w0 : w0 + wn], I_, gz2[:, w0 : w0 + wn],
                        start=False, stop=False,
                    )
                    nc.tensor.matmul(
                        ps[:, w0 : w0 + wn], I_, q_[:, w0 : w0 + wn],
                        start=False, stop=True,
                    )
                vsrc = ps
            else:  # mixed: ACT squares psum->psum, identities accumulate
                ps2 = psum.tile([128, sbc], f32, tag="ps2")
                nc.scalar.activation(ps2[:], ps[:], SQUARE)
                for w0, wn in MMW:
                    nc.tensor.matmul(
                        ps2[:, w0 : w0 + wn], I_, gx2[:, w0 : w0 + wn],
                        start=False, stop=False,
                    )
                    nc.tensor.matmul(
                        ps2[:, w0 : w0 + wn], I_, gz2[:, w0 : w0 + wn],
                        start=False, stop=True,
                    )
                vsrc = ps2
            # sqrt(v + eps) with fused row-sum accum
            pb = pbp.tile([128, spsb * W], f16, tag=f"pb{tk}")
            v3 = vsrc[:].rearrange("p (s w) -> p s w", s=spsb)
            pb3 = pb[:].rearrange("p (s w) -> p s w", s=spsb)
            slot = rep * nsb + sb
            nc.scalar.activation(
                pb3[:, :, :],
                v3[:, :, 2 : 2 + W],
                SQRT,
                bias=eps_t[:],
                accum_out=SA[f"s{tk}"][:, slot : slot + 1],
            )
            PBTILE[(tk, sb)] = pb
            # products
            if tk == 2:
                prod = work.tile([128, spsb * W], f16, tag="prod")
                nc.vector.scalar_tensor_tensor(
                    prod[:, :],
                    PBTILE[(1, sb)][:, :],
                    1.0,
                    pb[:, :],
                    op0=MULT,
                    op1=MULT,
                    accum_out=SA["pa"][:, slot : slot + 1],
                )
            elif tk == 3:
                pbr = work.tile([HB, spsb * W], f16, tag="pbr")
                nc.sync.dma_start(pbr[:, :], pb[HB:128, :])
                prod3 = work.tile([HB, spsb * W], f16, tag="prod3")
                nc.vector.scalar_tensor_tensor(
                    prod3[:, :],
                    pb[0:HB, :],
                    1.0,
                    pbr[:, :],
                    op0=MULT,
                    op1=MULT,
                    accum_out=SA["pb"][0:HB, slot : slot + 1],
                )

        pend = []
        for unit in units:
            pend.append(emit_p1(unit))
            if len(pend) > lag:
                emit_p2(pend.pop(0))
        for st_ in pend:
            emit_p2(st_)

        # reduce slots, write out
        for col, q in enumerate(("s1", "s2", "s3", "pa", "pb")):
            pc = HB if q == "pb" else 128
            r = accp.tile([128, 1], f32, tag=f"red{col}")
            nc.vector.tensor_reduce(r[0:pc, :], SA[q][0:pc, 0:nslots], AXX, ADD)
            nc.sync.dma_start(out[0:pc, col : col + 1], r[0:pc, :])

    return nc


def get_nc():
    if "nc" not in _NC_CACHE:
        _NC_CACHE["nc"] = build_nc()
    return _NC_CACHE["nc"]


def _dmat(k):
    d = np.zeros((k, k), np.float16)
    for m in range(k):
        if m + 1 < k:
            d[m + 1, m] = 1.0
        if m - 1 >= 0:
            d[m - 1, m] = -1.0
    return d


def _mats():
    m1 = _dmat(128)
    m3 = np.zeros((128, 128), np.float16)
    m3[0:HB, 0:HB] = _dmat(HB)
    m3[HB:128, HB:128] = _dmat(HB)
    b1 = np.zeros((128, 128), np.float16)
    b1[0, 127] = 1.0      # T1 gy[127] += x[128] (= T3 p0)
    b2 = np.zeros((128, 128), np.float16)
    b2[HB, 127] = 1.0     # T2 gy[127] += X[128] (= T3 p64)
    b3 = np.zeros((128, 128), np.float16)
    b3[127, 0] = -1.0     # T3 gy[h128] -= x[127] (= T1 p127)
    b4 = np.zeros((128, 128), np.float16)
    b4[127, HB] = -1.0    # T3 gy[h128,tgt] -= X[127] (= T2 p127)
    i_ = np.eye(128, dtype=np.float16)
    return np.concatenate([m1, m3, b1, b2, b3, b4, i_], axis=1)


MATS_NP = _mats()


def _shard(vol, q):
    sh = np.zeros((S, H, WP), np.float16)
    d0 = DL * q - 1
    lo, hi = max(d0, 0), min(d0 + S, DVOL)
    sh[lo - d0 : hi - d0, :, 2 : 2 + W] = vol[lo:hi].astype(np.float16)
    return np.ascontiguousarray(sh.transpose(1, 0, 2)).reshape(H, FREE)


def make_in_maps(pred, target):
    pred = np.asarray(pred, dtype=np.float32).reshape(BATCH, DVOL, H, W)
    target = np.asarray(target, dtype=np.float32).reshape(BATCH, DVOL, H, W)
    maps = []
    for c in range(NCORES):
        b, q = divmod(c, NQ)
        maps.append(
            {
                "xp": _shard(pred[b], q),
                "xt": _shard(target[b], q),
                "mats": MATS_NP,
            }
        )
    return maps


def combine(results):
    spt = pt = 0.0
    for r in results:
        o = np.asarray(r["o"], dtype=np.float64)
        spt += o[:, 0].sum() + o[:, 1].sum() + o[:, 2].sum()
        pt += o[:, 3].sum() + o[0:HB, 4].sum()
    dice = (2.0 * pt + EPS) / (spt + EPS)
    return np.float32(1.0 - dice)


def run_on_device(in_maps, **kwargs):
    from concourse.bass_utils import run_bass_kernel_spmd

    nc = get_nc()
    return run_bass_kernel_spmd(nc, in_maps, core_ids=list(range(NCORES)), **kwargs)


def kernel(pred, target):
    in_maps = make_in_maps(pred, target)
    res = run_on_device(in_maps)
    return combine(res.results)


if __name__ == "__main__":
    rng = np.random.default_rng(0)
    p = rng.random((2, 1, 192, 192, 192), np.float32)
    t = rng.random((2, 1, 192, 192, 192), np.float32)
    print(kernel(p, t))


# revision 3
# speedup vs baseline: 1.0949x; 1.0655x over previous
"""Kernel v3: 3-tile row packing — [pred h0:128], [tgt h0:128],
[pred h128:192 | tgt h128:192] — so every op uses all 128 partitions
(25% fewer columns than the 2-chunk layout) and all rows are valid.

Height-diff gy via per-tile shift matmuls; the 2-row cross-tile
boundaries (h=127/128 of each tensor) are stitched with extra matmuls
from the neighboring tile accumulated into the same PSUM. T3's
pb(pred)*tb(tgt) product needs lane alignment: DMA-realign rows 64:128
onto 0:64 then TSP. Dice needs only Σpb+Σtb so per-tile sqrt accums sum
host-side.
"""

import sys

sys.path.insert(0, "/opt/trn_rl_repo")

import numpy as np

BATCH = 2
DVOL = 192
H = 192
W = 192
NCORES = 8
NQ = 4
DL = DVOL // NQ      # 48
S = DL + 2           # 50
WP = W + 4           # 196
FREE = S * WP        # 9800
EPS = 1e-5
HB = 64              # T3 block height

_NC_CACHE = {}
_WAIT_CAP = 1


def _split_multiwait_json(bs: bytes) -> bytes:
    import json

    m = json.loads(bs)
    changed = False
    for fn in m.get("functions", []):
        for blk in fn.get("blocks", []):
            insts = blk.get("instructions")
            if not insts:
                continue
            out = []
            for ins in insts:
                si = ins.get("sync_info") or {}
                ow = si.get("on_wait") or []
                if len(ow) > _WAIT_CAP:
                    chunks = [
                        ow[i : i + _WAIT_CAP] for i in range(0, len(ow), _WAIT_CAP)
                    ]
                    for ci, ch in enumerate(chunks[:-1]):
                        out.append(
                            {
                                "debug": ins.get("debug", 0),
                                "engine": ins["engine"],
                                "ins": [],
                                "outs": [],
                                "is_reset_sema": False,
                                "name": f"{ins['name']}__w{ci}",
                                "opcode": "EventSemaphore",
                                "sync_info": {"on_update": [], "on_wait": ch},
                            }
                        )
                    si["on_wait"] = chunks[-1]
                    ins["sync_info"] = si
                    changed = True
                out.append(ins)
            blk["instructions"] = out
    if not changed:
        return bs
    return json.dumps(m).encode()


def _install_json_patch():
    import concourse.bass as bass

    if getattr(bass.Bass, "_bl_json_patched", False):
        return
    orig = bass.Bass.to_json_bytes

    def to_json_bytes(self, *a, **k):
        return _split_multiwait_json(orig(self, *a, **k))

    bass.Bass.to_json_bytes = to_json_bytes
    bass.Bass._bl_json_patched = True


def build_nc(
    repeats=1,
    sbsl=6,            # slices per sub-block
    vmode="psum_acc",  # "psum_acc" | "mixed"
    n_subpool=17,      # of 36 units: both subs on Pool (Bresenham-spread)
    n_gx2act=0,
    n_gz2act=0,
    lag=2,
    work_bufs=5,
    pb_bufs=3,
    dma_pieces=6,
):
    from contextlib import ExitStack

    import concourse.bass as bass
    import concourse.mybir as mybir
    from concourse import tile

    _install_json_patch()

    f16 = mybir.dt.float16
    f32 = mybir.dt.float32
    MULT = mybir.AluOpType.mult
    ADD = mybir.AluOpType.add
    SQUARE = mybir.ActivationFunctionType.Square
    SQRT = mybir.ActivationFunctionType.Sqrt
    AXX = mybir.AxisListType.X

    sbc = sbsl * WP
    OUTC = DL * WP
    nsb = OUTC // sbc
    assert nsb * sbc == OUTC
    MMW = []
    w0 = 0
    while w0 < sbc:
        MMW.append((w0, min(512, sbc - w0)))
        w0 += 512
    psum_banks = -(-sbc * 4 // 2048)   # per ps tile (unfused, fp32)
    if vmode == "mixed":
        psum_bufs = max(1, 8 // (2 * psum_banks))
    else:
        psum_bufs = min(4, max(2, 8 // psum_banks))

    nc = bass.Bass("TRN2", target_bir_lowering=False, debug=False)

    xp = nc.dram_tensor("xp", [H, FREE], f16, kind="ExternalInput")
    xt = nc.dram_tensor("xt", [H, FREE], f16, kind="ExternalInput")
    mats = nc.dram_tensor("mats", [128, 7 * 128], f16, kind="ExternalInput")
    out = nc.dram_tensor("o", [128, 5], f32, kind="ExternalOutput")

    with tile.TileContext(nc) as tc, ExitStack() as ctx:
        const = ctx.enter_context(tc.tile_pool(name="const", bufs=1))
        xpool = ctx.enter_context(tc.tile_pool(name="x", bufs=1))
        work = ctx.enter_context(tc.tile_pool(name="work", bufs=work_bufs))
        pbp = ctx.enter_context(tc.tile_pool(name="pb", bufs=pb_bufs))
        accp = ctx.enter_context(tc.tile_pool(name="acc", bufs=1))
        psum = ctx.enter_context(
            tc.tile_pool(name="psum", bufs=psum_bufs, space="PSUM")
        )

        mt = const.tile([128, 7 * 128], f16, tag="mats")
        nc.sync.dma_start(mt[:], mats[:, :])
        M1 = mt[:, 0:128]          # within-tile shift for T1/T2
        M3 = mt[:, 128:256]        # block-diag shift for T3
        B1 = mt[:, 256:384]        # T3 -> T1 psum (+x[128] to row 127)
        B2 = mt[:, 384:512]        # T3 -> T2 psum
        B3 = mt[:, 512:640]        # T1 -> T3 psum (-x[127] to row 0)
        B4 = mt[:, 640:768]        # T2 -> T3 psum (-X[127] to row 64)
        I_ = mt[:, 768:896]        # identity
        eps_t = const.tile([128, 1], f32, tag="eps")
        nc.vector.memset(eps_t[:], EPS)

        # tiles: T1 = pred rows 0:128, T2 = tgt rows 0:128,
        #        T3 = pred rows 128:192 on p0:64, tgt rows 128:192 on p64:128
        T = {}
        for k in (1, 2, 3):
            T[k] = xpool.tile([128, FREE], f16, tag=f"t{k}", name=f"t{k}")
        first = sbc + 392
        csteps = [(0, first)]
        cstep = -(-(FREE - first) // dma_pieces)
        c = first
        while c < FREE:
            csteps.append((c, min(FREE, c + cstep)))
            c += cstep
        for c0_, c1_ in csteps:
            nc.sync.dma_start(T[1][:, c0_:c1_], xp[0:128, c0_:c1_])
            nc.sync.dma_start(T[2][:, c0_:c1_], xt[0:128, c0_:c1_])
            nc.sync.dma_start(T[3][0:HB, c0_:c1_], xp[128:192, c0_:c1_])
            nc.sync.dma_start(T[3][HB:128, c0_:c1_], xt[128:192, c0_:c1_])

        # accum slots
        SA = {}
        nslots = nsb * repeats
        for q in ("s1", "s2", "s3", "pa", "pb"):
            SA[q] = accp.tile([128, nslots], f32, tag=f"sa_{q}", name=f"sa_{q}")

        spsb = sbsl
        nu = 3 * nsb
        kpool = max(0, round(n_subpool * nu / 36))
        kgx = max(0, round(n_gx2act * nu / 36))
        kgz = max(0, round(n_gz2act * nu / 36))
        ucount = [0]
        units = []
        for rep in range(repeats):
            for sb in range(nsb):
                for tk in (3, 1, 2):
                    units.append((rep, tk, sb))

        # per-(tile,sb) shift matmul plans: (main_mat, [(bnd_mat, src_tile)...])
        PBTILE = {}  # (tk, sb) -> pb tile (for products)

        def emit_p1(unit):
            rep, tk, sb = unit
            u = ucount[0]
            ucount[0] += 1
            x = T[tk]
            c0 = sb * sbc
            g = work.tile([128, 2, sbc], f16, tag="g")
            on_pool = kpool > 0 and ((u % nu) * kpool) % nu < kpool
            eng = nc.gpsimd if on_pool else nc.vector
            eng.tensor_sub(
                g[:, 0, :], x[:, c0 + 392 : c0 + 392 + sbc], x[:, c0 : c0 + sbc]
            )
            eng2 = nc.gpsimd if on_pool else nc.vector
            eng2.tensor_sub(
                g[:, 1, :],
                x[:, c0 + 197 : c0 + 197 + sbc],
                x[:, c0 + 195 : c0 + 195 + sbc],
            )
            g2 = work.tile([128, 2, sbc], f16, tag="g2")
            if kgx > 0 and ((u % nu) * kgx) % nu < kgx:
                nc.scalar.activation(g2[:], g[:], SQUARE)
            else:
                nc.vector.tensor_mul(g2[:], g[:], g[:])
            gx2 = g2[:, 0, :]
            gz2 = g2[:, 1, :]
            # gy into psum: main shift matmul + boundary stitches
            ps = psum.tile([128, sbc], f32, tag="ps")
            if tk == 1:
                plan = [(M1, T[1], True, False), (B1, T[3], False, True)]
            elif tk == 2:
                plan = [(M1, T[2], True, False), (B2, T[3], False, True)]
            else:
                plan = [
                    (M3, T[3], True, False),
                    (B3, T[1], False, False),
                    (B4, T[2], False, True),
                ]
            for w0, wn in MMW:
                for mat, src, st, sp in plan:
                    nc.tensor.matmul(
                        ps[:, w0 : w0 + wn],
                        mat,
                        src[:, 196 + c0 + w0 : 196 + c0 + w0 + wn],
                        start=st,
                        stop=sp,
                    )
            return (u, unit, c0, gx2, gz2, ps)

        def emit_p2(st_):
            u, unit, c0, gx2, gz2, ps = st_
            rep, tk, sb = unit
            if vmode == "psum_acc":
                q_ = work.tile([128, sbc], f16, tag="q")
                nc.scalar.activation(q_[:], ps[:], SQUARE)
                for w0, wn in MMW:
                    nc.tensor.matmul(
                        ps[:, w0 : w0 + wn], I_, gx2[:, w0 : w0 + wn],
                        start=True, stop=False,
                    )
                    nc.tensor.matmul(
                        ps[:, w0 : w0 + wn], I_, gz2[:, w0 : w0 + wn],
                        start=False, stop=False,
                    )
                    nc.tensor.matmul(
                        ps[:, w0 : w0 + wn], I_, q_[:, w0 : w0 + wn],
                        start=False, stop=True,
                    )
                vsrc = ps
            else:  # mixed: ACT squares psum->psum, identities accumulate
                ps2 = psum.tile([128, sbc], f32, tag="ps2")
                nc.scalar.activation(ps2[:], ps[:], SQUARE)
                for w0, wn in MMW:
                    nc.tensor.matmul(
                        ps2[:, w0 : w0 + wn], I_, gx2[:, w0 : w0 + wn],
                        start=False, stop=False,
                    )
                    nc.tensor.matmul(
                        ps2[:, w0 : w0 + wn], I_, gz2[:, w0 : w0 + wn],
                        start=False, stop=True,
                    )
                vsrc = ps2
            # sqrt(v + eps) with fused row-sum accum
            pb = pbp.tile([128, spsb * W], f16, tag=f"pb{tk}")
            v3 = vsrc[:].rearrange("p (s w) -> p s w", s=spsb)
            pb3 = pb[:].rearrange("p (s w) -> p s w", s=spsb)
            slot = rep * nsb + sb
            nc.scalar.activation(
                pb3[:, :, :],
                v3[:, :, 2 : 2 + W],
                SQRT,
                bias=eps_t[:],
                accum_out=SA[f"s{tk}"][:, slot : slot + 1],
            )
            PBTILE[(tk, sb)] = pb
            # products
            if tk == 2:
                prod = work.tile([128, spsb * W], f16, tag="prod")
                nc.vector.scalar_tensor_tensor(
                    prod[:, :],
                    PBTILE[(1, sb)][:, :],
                    1.0,
                    pb[:, :],
                    op0=MULT,
                    op1=MULT,
                    accum_out=SA["pa"][:, slot : slot + 1],
                )
            elif tk == 3:
                pbr = work.tile([HB, spsb * W], f16, tag="pbr")
                nc.sync.dma_start(pbr[:, :], pb[HB:128, :])
                prod3 = work.tile([HB, spsb * W], f16, tag="prod3")
                nc.vector.scalar_tensor_tensor(
                    prod3[:, :],
                    pb[0:HB, :],
                    1.0,
                    pbr[:, :],
                    op0=MULT,
                    op1=MULT,
                    accum_out=SA["pb"][0:HB, slot : slot + 1],
                )

        pend = []
        for unit in units:
            pend.append(emit_p1(unit))
            if len(pend) > lag:
                emit_p2(pend.pop(0))
        for st_ in pend:
            emit_p2(st_)

        # reduce slots, write out
        for col, q in enumerate(("s1", "s2", "s3", "pa", "pb")):
            pc = HB if q == "pb" else 128
            r = accp.tile([128, 1], f32, tag=f"red{col}")
            nc.vector.tensor_reduce(r[0:pc, :], SA[q][0:pc, 0:nslots], AXX, ADD)
            nc.sync.dma_start(out[0:pc, col : col + 1], r[0:pc, :])

    return nc


def get_nc():
    if "nc" not in _NC_CACHE:
        _NC_CACHE["nc"] = build_nc()
    return _NC_CACHE["nc"]


def _dmat(k):
    d = np.zeros((k, k), np.float16)
    for m in range(k):
        if m + 1 < k:
            d[m + 1, m] = 1.0
        if m - 1 >= 0:
            d[m - 1, m] = -1.0
    return d


def _mats():
    m1 = _dmat(128)
    m3 = np.zeros((128, 128), np.float16)
    m3[0:HB, 0:HB] = _dmat(HB)
    m3[HB:128, HB:128] = _dmat(HB)
    b1 = np.zeros((128, 128), np.float16)
    b1[0, 127] = 1.0      # T1 gy[127] += x[128] (= T3 p0)
    b2 = np.zeros((128, 128), np.float16)
    b2[HB, 127] = 1.0     # T2 gy[127] += X[128] (= T3 p64)
    b3 = np.zeros((128, 128), np.float16)
    b3[127, 0] = -1.0     # T3 gy[h128] -= x[127] (= T1 p127)
    b4 = np.zeros((128, 128), np.float16)
    b4[127, HB] = -1.0    # T3 gy[h128,tgt] -= X[127] (= T2 p127)
    i_ = np.eye(128, dtype=np.float16)
    return np.concatenate([m1, m3, b1, b2, b3, b4, i_], axis=1)


MATS_NP = _mats()


def _shard(vol, q):
    sh = np.zeros((S, H, WP), np.float16)
    d0 = DL * q - 1
    lo, hi = max(d0, 0), min(d0 + S, DVOL)
    sh[lo - d0 : hi - d0, :, 2 : 2 + W] = vol[lo:hi].astype(np.float16)
    return np.ascontiguousarray(sh.transpose(1, 0, 2)).reshape(H, FREE)


def make_in_maps(pred, target):
    pred = np.asarray(pred, dtype=np.float32).reshape(BATCH, DVOL, H, W)
    target = np.asarray(target, dtype=np.float32).reshape(BATCH, DVOL, H, W)
    maps = []
    for c in range(NCORES):
        b, q = divmod(c, NQ)
        maps.append(
            {
                "xp": _shard(pred[b], q),
                "xt": _shard(target[b], q),
                "mats": MATS_NP,
            }
        )
    return maps


def combine(results):
    spt = pt = 0.0
    for r in results:
        o = np.asarray(r["o"], dtype=np.float64)
        spt += o[:, 0].sum() + o[:, 1].sum() + o[:, 2].sum()
        pt += o[:, 3].sum() + o[0:HB, 4].sum()
    dice = (2.0 * pt + EPS) / (spt + EPS)
    return np.float32(1.0 - dice)


def run_on_device(in_maps, **kwargs):
    from concourse.bass_utils import run_bass_kernel_spmd

    nc = get_nc()
    return run_bass_kernel_spmd(nc, in_maps, core_ids=list(range(NCORES)), **kwargs)


def kernel(pred, target):
    in_maps = make_in_maps(pred, target)
    res = run_on_device(in_maps)
    return combine(res.results)


if __name__ == "__main__":
    rng = np.random.default_rng(0)
    p = rng.random((2, 1, 192, 192, 192), np.float32)
    t = rng.random((2, 1, 192, 192, 192), np.float32)
    print(kernel(p, t))


# revision 4
# speedup vs baseline: 1.3523x; 1.2351x over previous
"""Kernel v3: 3-tile row packing — [pred h0:128], [tgt h0:128],
[pred h128:192 | tgt h128:192] — so every op uses all 128 partitions
(25% fewer columns than the 2-chunk layout) and all rows are valid.

Height-diff gy via per-tile shift matmuls; the 2-row cross-tile
boundaries (h=127/128 of each tensor) are stitched with extra matmuls
from the neighboring tile accumulated into the same PSUM. T3's
pb(pred)*tb(tgt) product needs lane alignment: DMA-realign rows 64:128
onto 0:64 then TSP. Dice needs only Σpb+Σtb so per-tile sqrt accums sum
host-side.
"""

import sys

sys.path.insert(0, "/opt/trn_rl_repo")

import numpy as np

BATCH = 2
DVOL = 192
H = 192
W = 192
NCORES = 8
NQ = 4
DL = DVOL // NQ      # 48
S = DL + 2           # 50
WP = W + 4           # 196
FREE = S * WP        # 9800
EPS = 1e-5
HB = 64              # T3 block height

_NC_CACHE = {}
_WAIT_CAP = 1


def _split_multiwait_json(bs: bytes) -> bytes:
    import json

    m = json.loads(bs)
    changed = False
    for fn in m.get("functions", []):
        for blk in fn.get("blocks", []):
            insts = blk.get("instructions")
            if not insts:
                continue
            out = []
            for ins in insts:
                si = ins.get("sync_info") or {}
                ow = si.get("on_wait") or []
                if len(ow) > _WAIT_CAP:
                    chunks = [
                        ow[i : i + _WAIT_CAP] for i in range(0, len(ow), _WAIT_CAP)
                    ]
                    for ci, ch in enumerate(chunks[:-1]):
                        out.append(
                            {
                                "debug": ins.get("debug", 0),
                                "engine": ins["engine"],
                                "ins": [],
                                "outs": [],
                                "is_reset_sema": False,
                                "name": f"{ins['name']}__w{ci}",
                                "opcode": "EventSemaphore",
                                "sync_info": {"on_update": [], "on_wait": ch},
                            }
                        )
                    si["on_wait"] = chunks[-1]
                    ins["sync_info"] = si
                    changed = True
                out.append(ins)
            blk["instructions"] = out
    if not changed:
        return bs
    return json.dumps(m).encode()


def _install_json_patch():
    import concourse.bass as bass

    if getattr(bass.Bass, "_bl_json_patched", False):
        return
    orig = bass.Bass.to_json_bytes

    def to_json_bytes(self, *a, **k):
        return _split_multiwait_json(orig(self, *a, **k))

    bass.Bass.to_json_bytes = to_json_bytes
    bass.Bass._bl_json_patched = True


def build_nc(
    repeats=1,
    sbsl=6,            # slices per sub-block
    vmode="psum_acc",  # "psum_acc" | "mixed"
    n_subpool=0,       # subs on Pool count — HW: gpsimd far slower than modeled, keep 0
    n_gx2act=0,
    n_gz2act=0,
    lag=2,
    work_bufs=5,
    pb_bufs=3,
    dma_pieces=6,
):
    from contextlib import ExitStack

    import concourse.bass as bass
    import concourse.mybir as mybir
    from concourse import tile

    _install_json_patch()

    f16 = mybir.dt.float16
    f32 = mybir.dt.float32
    MULT = mybir.AluOpType.mult
    ADD = mybir.AluOpType.add
    SQUARE = mybir.ActivationFunctionType.Square
    SQRT = mybir.ActivationFunctionType.Sqrt
    AXX = mybir.AxisListType.X

    sbc = sbsl * WP
    OUTC = DL * WP
    nsb = OUTC // sbc
    assert nsb * sbc == OUTC
    MMW = []
    w0 = 0
    while w0 < sbc:
        MMW.append((w0, min(512, sbc - w0)))
        w0 += 512
    psum_banks = -(-sbc * 4 // 2048)   # per ps tile (unfused, fp32)
    if vmode == "mixed":
        psum_bufs = max(1, 8 // (2 * psum_banks))
    else:
        psum_bufs = min(4, max(2, 8 // psum_banks))

    nc = bass.Bass("TRN2", target_bir_lowering=False, debug=False)

    xp = nc.dram_tensor("xp", [H, FREE], f16, kind="ExternalInput")
    xt = nc.dram_tensor("xt", [H, FREE], f16, kind="ExternalInput")
    mats = nc.dram_tensor("mats", [128, 7 * 128], f16, kind="ExternalInput")
    nslots_ = DL * WP // (sbsl * WP)
    out = nc.dram_tensor("o", [128, 5 * nslots_], f32, kind="ExternalOutput")

    with tile.TileContext(nc) as tc, ExitStack() as ctx:
        const = ctx.enter_context(tc.tile_pool(name="const", bufs=1))
        xpool = ctx.enter_context(tc.tile_pool(name="x", bufs=1))
        work = ctx.enter_context(tc.tile_pool(name="work", bufs=work_bufs))
        pbp = ctx.enter_context(tc.tile_pool(name="pb", bufs=pb_bufs))
        accp = ctx.enter_context(tc.tile_pool(name="acc", bufs=1))
        psum = ctx.enter_context(
            tc.tile_pool(name="psum", bufs=psum_bufs, space="PSUM")
        )

        mt = const.tile([128, 7 * 128], f16, tag="mats")
        nc.sync.dma_start(mt[:], mats[:, :])
        M1 = mt[:, 0:128]          # within-tile shift for T1/T2
        M3 = mt[:, 128:256]        # block-diag shift for T3
        B1 = mt[:, 256:384]        # T3 -> T1 psum (+x[128] to row 127)
        B2 = mt[:, 384:512]        # T3 -> T2 psum
        B3 = mt[:, 512:640]        # T1 -> T3 psum (-x[127] to row 0)
        B4 = mt[:, 640:768]        # T2 -> T3 psum (-X[127] to row 64)
        I_ = mt[:, 768:896]        # identity
        eps_t = const.tile([128, 1], f32, tag="eps")
        nc.vector.memset(eps_t[:], EPS)

        # tiles: T1 = pred rows 0:128, T2 = tgt rows 0:128,
        #        T3 = pred rows 128:192 on p0:64, tgt rows 128:192 on p64:128
        T = {}
        for k in (1, 2, 3):
            T[k] = xpool.tile([128, FREE], f16, tag=f"t{k}", name=f"t{k}")
        first = sbc + 392
        csteps = [(0, first)]
        cstep = -(-(FREE - first) // dma_pieces)
        c = first
        while c < FREE:
            csteps.append((c, min(FREE, c + cstep)))
            c += cstep
        for c0_, c1_ in csteps:
            nc.sync.dma_start(T[1][:, c0_:c1_], xp[0:128, c0_:c1_])
            nc.sync.dma_start(T[2][:, c0_:c1_], xt[0:128, c0_:c1_])
            nc.sync.dma_start(T[3][0:HB, c0_:c1_], xp[128:192, c0_:c1_])
            nc.sync.dma_start(T[3][HB:128, c0_:c1_], xt[128:192, c0_:c1_])

        # accum slots
        SA = {}
        nslots = nsb
        for q in ("s1", "s2", "s3", "pa", "pb"):
            SA[q] = accp.tile([128, nslots], f32, tag=f"sa_{q}", name=f"sa_{q}")

        spsb = sbsl
        nu = 3 * nsb
        kpool = max(0, round(n_subpool * nu / 36))
        kgx = max(0, round(n_gx2act * nu / 36))
        kgz = max(0, round(n_gz2act * nu / 36))
        ucount = [0]
        units = []
        for rep in range(repeats):
            for sb in range(nsb):
                for tk in (3, 1, 2):
                    units.append((rep, tk, sb))

        # per-(tile,sb) shift matmul plans: (main_mat, [(bnd_mat, src_tile)...])
        PBTILE = {}  # (tk, sb) -> pb tile (for products)

        def emit_p1(unit):
            rep, tk, sb = unit
            u = ucount[0]
            ucount[0] += 1
            x = T[tk]
            c0 = sb * sbc
            g = work.tile([128, 2, sbc], f16, tag="g")
            on_pool = kpool > 0 and ((u % nu) * kpool) % nu < kpool
            eng = nc.gpsimd if on_pool else nc.vector
            eng.tensor_sub(
                g[:, 0, :], x[:, c0 + 392 : c0 + 392 + sbc], x[:, c0 : c0 + sbc]
            )
            eng2 = nc.gpsimd if on_pool else nc.vector
            eng2.tensor_sub(
                g[:, 1, :],
                x[:, c0 + 197 : c0 + 197 + sbc],
                x[:, c0 + 195 : c0 + 195 + sbc],
            )
            g2 = work.tile([128, 2, sbc], f16, tag="g2")
            if kgx > 0 and ((u % nu) * kgx) % nu < kgx:
                nc.scalar.activation(g2[:], g[:], SQUARE)
            else:
                nc.vector.tensor_mul(g2[:], g[:], g[:])
            gx2 = g2[:, 0, :]
            gz2 = g2[:, 1, :]
            # gy into psum: main shift matmul + boundary stitches
            ps = psum.tile([128, sbc], f32, tag="ps")
            if tk == 1:
                plan = [(M1, T[1], True, False), (B1, T[3], False, True)]
            elif tk == 2:
                plan = [(M1, T[2], True, False), (B2, T[3], False, True)]
            else:
                plan = [
                    (M3, T[3], True, False),
                    (B3, T[1], False, False),
                    (B4, T[2], False, True),
                ]
            for w0, wn in MMW:
                for mat, src, st, sp in plan:
                    nc.tensor.matmul(
                        ps[:, w0 : w0 + wn],
                        mat,
                        src[:, 196 + c0 + w0 : 196 + c0 + w0 + wn],
                        start=st,
                        stop=sp,
                    )
            return (u, unit, c0, gx2, gz2, ps)

        def emit_p2(st_):
            u, unit, c0, gx2, gz2, ps = st_
            rep, tk, sb = unit
            if vmode == "psum_acc":
                q_ = work.tile([128, sbc], f16, tag="q")
                nc.scalar.activation(q_[:], ps[:], SQUARE)
                for w0, wn in MMW:
                    nc.tensor.matmul(
                        ps[:, w0 : w0 + wn], I_, gx2[:, w0 : w0 + wn],
                        start=True, stop=False,
                    )
                    nc.tensor.matmul(
                        ps[:, w0 : w0 + wn], I_, gz2[:, w0 : w0 + wn],
                        start=False, stop=False,
                    )
                    nc.tensor.matmul(
                        ps[:, w0 : w0 + wn], I_, q_[:, w0 : w0 + wn],
                        start=False, stop=True,
                    )
                vsrc = ps
            else:  # mixed: ACT squares psum->psum, identities accumulate
                ps2 = psum.tile([128, sbc], f32, tag="ps2")
                nc.scalar.activation(ps2[:], ps[:], SQUARE)
                for w0, wn in MMW:
                    nc.tensor.matmul(
                        ps2[:, w0 : w0 + wn], I_, gx2[:, w0 : w0 + wn],
                        start=False, stop=False,
                    )
                    nc.tensor.matmul(
                        ps2[:, w0 : w0 + wn], I_, gz2[:, w0 : w0 + wn],
                        start=False, stop=True,
                    )
                vsrc = ps2
            # sqrt(v + eps) with fused row-sum accum
            pb = pbp.tile([128, spsb * W], f16, tag=f"pb{tk}")
            v3 = vsrc[:].rearrange("p (s w) -> p s w", s=spsb)
            pb3 = pb[:].rearrange("p (s w) -> p s w", s=spsb)
            slot = sb
            nc.scalar.activation(
                pb3[:, :, :],
                v3[:, :, 2 : 2 + W],
                SQRT,
                bias=eps_t[:],
                accum_out=SA[f"s{tk}"][:, slot : slot + 1],
            )
            PBTILE[(tk, sb)] = pb
            # products
            if tk == 2:
                prod = work.tile([128, spsb * W], f16, tag="prod")
                nc.vector.scalar_tensor_tensor(
                    prod[:, :],
                    PBTILE[(1, sb)][:, :],
                    1.0,
                    pb[:, :],
                    op0=MULT,
                    op1=MULT,
                    accum_out=SA["pa"][:, slot : slot + 1],
                )
            elif tk == 3:
                pbr = work.tile([HB, spsb * W], f16, tag="pbr")
                nc.sync.dma_start(pbr[:, :], pb[HB:128, :])
                prod3 = work.tile([HB, spsb * W], f16, tag="prod3")
                nc.vector.scalar_tensor_tensor(
                    prod3[:, :],
                    pb[0:HB, :],
                    1.0,
                    pbr[:, :],
                    op0=MULT,
                    op1=MULT,
                    accum_out=SA["pb"][0:HB, slot : slot + 1],
                )

        pend = []
        for unit in units:
            pend.append(emit_p1(unit))
            if len(pend) > lag:
                emit_p2(pend.pop(0))
        for st_ in pend:
            emit_p2(st_)

        # ship raw accum slots; host reduces (saves tail reduces+DMAs)
        for col, q in enumerate(("s1", "s2", "s3", "pa", "pb")):
            pc = HB if q == "pb" else 128
            nc.sync.dma_start(
                out[0:pc, col * nslots : (col + 1) * nslots],
                SA[q][0:pc, 0:nslots],
            )

    return nc


def get_nc():
    if "nc" not in _NC_CACHE:
        _NC_CACHE["nc"] = build_nc()
    return _NC_CACHE["nc"]


def _dmat(k):
    d = np.zeros((k, k), np.float16)
    for m in range(k):
        if m + 1 < k:
            d[m + 1, m] = 1.0
        if m - 1 >= 0:
            d[m - 1, m] = -1.0
    return d


def _mats():
    m1 = _dmat(128)
    m3 = np.zeros((128, 128), np.float16)
    m3[0:HB, 0:HB] = _dmat(HB)
    m3[HB:128, HB:128] = _dmat(HB)
    b1 = np.zeros((128, 128), np.float16)
    b1[0, 127] = 1.0      # T1 gy[127] += x[128] (= T3 p0)
    b2 = np.zeros((128, 128), np.float16)
    b2[HB, 127] = 1.0     # T2 gy[127] += X[128] (= T3 p64)
    b3 = np.zeros((128, 128), np.float16)
    b3[127, 0] = -1.0     # T3 gy[h128] -= x[127] (= T1 p127)
    b4 = np.zeros((128, 128), np.float16)
    b4[127, HB] = -1.0    # T3 gy[h128,tgt] -= X[127] (= T2 p127)
    i_ = np.eye(128, dtype=np.float16)
    return np.concatenate([m1, m3, b1, b2, b3, b4, i_], axis=1)


MATS_NP = _mats()


def _shard(vol, q):
    sh = np.zeros((S, H, WP), np.float16)
    d0 = DL * q - 1
    lo, hi = max(d0, 0), min(d0 + S, DVOL)
    sh[lo - d0 : hi - d0, :, 2 : 2 + W] = vol[lo:hi].astype(np.float16)
    return np.ascontiguousarray(sh.transpose(1, 0, 2)).reshape(H, FREE)


def make_in_maps(pred, target):
    pred = np.asarray(pred, dtype=np.float32).reshape(BATCH, DVOL, H, W)
    target = np.asarray(target, dtype=np.float32).reshape(BATCH, DVOL, H, W)
    maps = []
    for c in range(NCORES):
        b, q = divmod(c, NQ)
        maps.append(
            {
                "xp": _shard(pred[b], q),
                "xt": _shard(target[b], q),
                "mats": MATS_NP,
            }
        )
    return maps


def combine(results):
    spt = pt = 0.0
    for r in results:
        o = np.asarray(r["o"], dtype=np.float64)
        ns = o.shape[1] // 5
        spt += o[:, 0 : 3 * ns].sum()
        pt += o[:, 3 * ns : 4 * ns].sum() + o[0:HB, 4 * ns : 5 * ns].sum()
    dice = (2.0 * pt + EPS) / (spt + EPS)
    return np.float32(1.0 - dice)


def run_on_device(in_maps, **kwargs):
    from concourse.bass_utils import run_bass_kernel_spmd

    nc = get_nc()
    return run_bass_kernel_spmd(nc, in_maps, core_ids=list(range(NCORES)), **kwargs)


def kernel(pred, target):
    in_maps = make_in_maps(pred, target)
    res = run_on_device(in_maps)
    return combine(res.results)


if __name__ == "__main__":
    rng = np.random.default_rng(0)
    p = rng.random((2, 1, 192, 192, 192), np.float32)
    t = rng.random((2, 1, 192, 192, 192), np.float32)
    print(kernel(p, t))


# revision 5
# speedup vs baseline: 1.7251x; 1.2757x over previous
"""Kernel v3: 3-tile row packing — [pred h0:128], [tgt h0:128],
[pred h128:192 | tgt h128:192] — so every op uses all 128 partitions
(25% fewer columns than the 2-chunk layout) and all rows are valid.

Height-diff gy via per-tile shift matmuls; the 2-row cross-tile
boundaries (h=127/128 of each tensor) are stitched with extra matmuls
from the neighboring tile accumulated into the same PSUM. T3's
pb(pred)*tb(tgt) product needs lane alignment: DMA-realign rows 64:128
onto 0:64 then TSP. Dice needs only Σpb+Σtb so per-tile sqrt accums sum
host-side.
"""

import sys

sys.path.insert(0, "/opt/trn_rl_repo")

import numpy as np

BATCH = 2
DVOL = 192
H = 192
W = 192
NCORES = 8
NQ = 4
DL = DVOL // NQ      # 48
S = DL + 2           # 50
WP = W + 4           # 196
FREE = S * WP        # 9800
EPS = 1e-5
HB = 64              # T3 block height

_NC_CACHE = {}
_WAIT_CAP = 1


def _split_multiwait_json(bs: bytes) -> bytes:
    import json

    m = json.loads(bs)
    changed = False
    for fn in m.get("functions", []):
        for blk in fn.get("blocks", []):
            insts = blk.get("instructions")
            if not insts:
                continue
            out = []
            for ins in insts:
                si = ins.get("sync_info") or {}
                ow = si.get("on_wait") or []
                if len(ow) > _WAIT_CAP:
                    chunks = [
                        ow[i : i + _WAIT_CAP] for i in range(0, len(ow), _WAIT_CAP)
                    ]
                    for ci, ch in enumerate(chunks[:-1]):
                        out.append(
                            {
                                "debug": ins.get("debug", 0),
                                "engine": ins["engine"],
                                "ins": [],
                                "outs": [],
                                "is_reset_sema": False,
                                "name": f"{ins['name']}__w{ci}",
                                "opcode": "EventSemaphore",
                                "sync_info": {"on_update": [], "on_wait": ch},
                            }
                        )
                    si["on_wait"] = chunks[-1]
                    ins["sync_info"] = si
                    changed = True
                out.append(ins)
            blk["instructions"] = out
    if not changed:
        return bs
    return json.dumps(m).encode()


def _install_json_patch():
    import concourse.bass as bass

    if getattr(bass.Bass, "_bl_json_patched", False):
        return
    orig = bass.Bass.to_json_bytes

    def to_json_bytes(self, *a, **k):
        return _split_multiwait_json(orig(self, *a, **k))

    bass.Bass.to_json_bytes = to_json_bytes
    bass.Bass._bl_json_patched = True


def build_nc(
    repeats=1,
    sbsl=6,            # slices per sub-block
    vmode="psum_acc",  # "psum_acc" | "mixed"
    n_subpool=0,       # subs on Pool count — HW: gpsimd far slower than modeled, keep 0
    n_gx2act=4,        # fused squares on ACT for ~3/24 units (HW-swept optimum)
    n_gz2act=0,
    lag=2,
    work_bufs=5,
    pb_bufs=3,
    dma_pieces=6,
):
    from contextlib import ExitStack

    import concourse.bass as bass
    import concourse.mybir as mybir
    from concourse import tile

    _install_json_patch()

    f16 = mybir.dt.float16
    f32 = mybir.dt.float32
    MULT = mybir.AluOpType.mult
    ADD = mybir.AluOpType.add
    SQUARE = mybir.ActivationFunctionType.Square
    SQRT = mybir.ActivationFunctionType.Sqrt
    AXX = mybir.AxisListType.X

    sbc = sbsl * WP
    OUTC = DL * WP
    nsb = OUTC // sbc
    assert nsb * sbc == OUTC
    MMW = []
    w0 = 0
    while w0 < sbc:
        MMW.append((w0, min(512, sbc - w0)))
        w0 += 512
    psum_banks = -(-sbc * 4 // 2048)   # per ps tile (unfused, fp32)
    if vmode == "mixed":
        psum_bufs = max(1, 8 // (2 * psum_banks))
    else:
        psum_bufs = min(4, max(2, 8 // psum_banks))

    nc = bass.Bass("TRN2", target_bir_lowering=False, debug=False)

    xp = nc.dram_tensor("xp", [H, FREE], f16, kind="ExternalInput")
    xt = nc.dram_tensor("xt", [H, FREE], f16, kind="ExternalInput")
    mats = nc.dram_tensor("mats", [128, 7 * 128], f16, kind="ExternalInput")
    nslots_ = DL * WP // (sbsl * WP)
    out = nc.dram_tensor("o", [128, 5 * nslots_], f32, kind="ExternalOutput")

    with tile.TileContext(nc) as tc, ExitStack() as ctx:
        const = ctx.enter_context(tc.tile_pool(name="const", bufs=1))
        xpool = ctx.enter_context(tc.tile_pool(name="x", bufs=1))
        work = ctx.enter_context(tc.tile_pool(name="work", bufs=work_bufs))
        pbp = ctx.enter_context(tc.tile_pool(name="pb", bufs=pb_bufs))
        accp = ctx.enter_context(tc.tile_pool(name="acc", bufs=1))
        psum = ctx.enter_context(
            tc.tile_pool(name="psum", bufs=psum_bufs, space="PSUM")
        )

        mt = const.tile([128, 7 * 128], f16, tag="mats")
        nc.sync.dma_start(mt[:], mats[:, :])
        M1 = mt[:, 0:128]          # within-tile shift for T1/T2
        M3 = mt[:, 128:256]        # block-diag shift for T3
        B1 = mt[:, 256:384]        # T3 -> T1 psum (+x[128] to row 127)
        B2 = mt[:, 384:512]        # T3 -> T2 psum
        B3 = mt[:, 512:640]        # T1 -> T3 psum (-x[127] to row 0)
        B4 = mt[:, 640:768]        # T2 -> T3 psum (-X[127] to row 64)
        I_ = mt[:, 768:896]        # identity
        eps_t = const.tile([128, 1], f32, tag="eps")
        nc.vector.memset(eps_t[:], EPS)

        # tiles: T1 = pred rows 0:128, T2 = tgt rows 0:128,
        #        T3 = pred rows 128:192 on p0:64, tgt rows 128:192 on p64:128
        T = {}
        for k in (1, 2, 3):
            T[k] = xpool.tile([128, FREE], f16, tag=f"t{k}", name=f"t{k}")
        first = sbc + 392
        csteps = [(0, first)]
        cstep = -(-(FREE - first) // dma_pieces)
        c = first
        while c < FREE:
            csteps.append((c, min(FREE, c + cstep)))
            c += cstep
        for c0_, c1_ in csteps:
            nc.sync.dma_start(T[1][:, c0_:c1_], xp[0:128, c0_:c1_])
            nc.sync.dma_start(T[2][:, c0_:c1_], xt[0:128, c0_:c1_])
            nc.sync.dma_start(T[3][0:HB, c0_:c1_], xp[128:192, c0_:c1_])
            nc.sync.dma_start(T[3][HB:128, c0_:c1_], xt[128:192, c0_:c1_])

        # accum slots
        SA = {}
        nslots = nsb
        for q in ("s1", "s2", "s3", "pa", "pb"):
            SA[q] = accp.tile([128, nslots], f32, tag=f"sa_{q}", name=f"sa_{q}")

        spsb = sbsl
        nu = 3 * nsb
        kpool = max(0, round(n_subpool * nu / 36))
        kgx = max(0, round(n_gx2act * nu / 36))
        kgz = max(0, round(n_gz2act * nu / 36))
        ucount = [0]
        units = []
        for rep in range(repeats):
            for sb in range(nsb):
                for tk in (3, 1, 2):
                    units.append((rep, tk, sb))

        # per-(tile,sb) shift matmul plans: (main_mat, [(bnd_mat, src_tile)...])
        PBTILE = {}  # (tk, sb) -> pb tile (for products)

        def emit_p1(unit):
            rep, tk, sb = unit
            u = ucount[0]
            ucount[0] += 1
            x = T[tk]
            c0 = sb * sbc
            g = work.tile([128, 2, sbc], f16, tag="g")
            on_pool = kpool > 0 and ((u % nu) * kpool) % nu < kpool
            eng = nc.gpsimd if on_pool else nc.vector
            eng.tensor_sub(
                g[:, 0, :], x[:, c0 + 392 : c0 + 392 + sbc], x[:, c0 : c0 + sbc]
            )
            eng2 = nc.gpsimd if on_pool else nc.vector
            eng2.tensor_sub(
                g[:, 1, :],
                x[:, c0 + 197 : c0 + 197 + sbc],
                x[:, c0 + 195 : c0 + 195 + sbc],
            )
            g2 = work.tile([128, 2, sbc], f16, tag="g2")
            if kgx > 0 and ((u % nu) * kgx) % nu < kgx:
                nc.scalar.activation(g2[:], g[:], SQUARE)
            else:
                nc.vector.tensor_mul(g2[:], g[:], g[:])
            gx2 = g2[:, 0, :]
            gz2 = g2[:, 1, :]
            # gy into psum: main shift matmul + boundary stitches
            ps = psum.tile([128, sbc], f32, tag="ps")
            if tk == 1:
                plan = [(M1, T[1], True, False), (B1, T[3], False, True)]
            elif tk == 2:
                plan = [(M1, T[2], True, False), (B2, T[3], False, True)]
            else:
                plan = [
                    (M3, T[3], True, False),
                    (B3, T[1], False, False),
                    (B4, T[2], False, True),
                ]
            for w0, wn in MMW:
                for mat, src, st, sp in plan:
                    nc.tensor.matmul(
                        ps[:, w0 : w0 + wn],
                        mat,
                        src[:, 196 + c0 + w0 : 196 + c0 + w0 + wn],
                        start=st,
                        stop=sp,
                    )
            return (u, unit, c0, gx2, gz2, ps)

        def emit_p2(st_):
            u, unit, c0, gx2, gz2, ps = st_
            rep, tk, sb = unit
            if vmode == "psum_acc":
                q_ = work.tile([128, sbc], f16, tag="q")
                nc.scalar.activation(q_[:], ps[:], SQUARE)
                for w0, wn in MMW:
                    nc.tensor.matmul(
                        ps[:, w0 : w0 + wn], I_, gx2[:, w0 : w0 + wn],
                        start=True, stop=False,
                    )
                    nc.tensor.matmul(
                        ps[:, w0 : w0 + wn], I_, gz2[:, w0 : w0 + wn],
                        start=False, stop=False,
                    )
                    nc.tensor.matmul(
                        ps[:, w0 : w0 + wn], I_, q_[:, w0 : w0 + wn],
                        start=False, stop=True,
                    )
                vsrc = ps
            else:  # mixed: ACT squares psum->psum, identities accumulate
                ps2 = psum.tile([128, sbc], f32, tag="ps2")
                nc.scalar.activation(ps2[:], ps[:], SQUARE)
                for w0, wn in MMW:
                    nc.tensor.matmul(
                        ps2[:, w0 : w0 + wn], I_, gx2[:, w0 : w0 + wn],
                        start=False, stop=False,
                    )
                    nc.tensor.matmul(
                        ps2[:, w0 : w0 + wn], I_, gz2[:, w0 : w0 + wn],
                        start=False, stop=True,
                    )
                vsrc = ps2
            # sqrt(v + eps) with fused row-sum accum
            pb = pbp.tile([128, spsb * W], f16, tag=f"pb{tk}")
            v3 = vsrc[:].rearrange("p (s w) -> p s w", s=spsb)
            pb3 = pb[:].rearrange("p (s w) -> p s w", s=spsb)
            slot = sb
            nc.scalar.activation(
                pb3[:, :, :],
                v3[:, :, 2 : 2 + W],
                SQRT,
                bias=eps_t[:],
                accum_out=SA[f"s{tk}"][:, slot : slot + 1],
            )
            PBTILE[(tk, sb)] = pb
            # products
            if tk == 2:
                prod = work.tile([128, spsb * W], f16, tag="prod")
                nc.vector.scalar_tensor_tensor(
                    prod[:, :],
                    PBTILE[(1, sb)][:, :],
                    1.0,
                    pb[:, :],
                    op0=MULT,
                    op1=MULT,
                    accum_out=SA["pa"][:, slot : slot + 1],
                )
            elif tk == 3:
                pbr = work.tile([HB, spsb * W], f16, tag="pbr")
                nc.sync.dma_start(pbr[:, :], pb[HB:128, :])
                prod3 = work.tile([HB, spsb * W], f16, tag="prod3")
                nc.vector.scalar_tensor_tensor(
                    prod3[:, :],
                    pb[0:HB, :],
                    1.0,
                    pbr[:, :],
                    op0=MULT,
                    op1=MULT,
                    accum_out=SA["pb"][0:HB, slot : slot + 1],
                )

        pend = []
        for unit in units:
            pend.append(emit_p1(unit))
            if len(pend) > lag:
                emit_p2(pend.pop(0))
        for st_ in pend:
            emit_p2(st_)

        # ship raw accum slots; host reduces (saves tail reduces+DMAs)
        for col, q in enumerate(("s1", "s2", "s3", "pa", "pb")):
            pc = HB if q == "pb" else 128
            nc.sync.dma_start(
                out[0:pc, col * nslots : (col + 1) * nslots],
                SA[q][0:pc, 0:nslots],
            )

    return nc


def get_nc():
    if "nc" not in _NC_CACHE:
        _NC_CACHE["nc"] = build_nc()
    return _NC_CACHE["nc"]


def _dmat(k):
    d = np.zeros((k, k), np.float16)
    for m in range(k):
        if m + 1 < k:
            d[m + 1, m] = 1.0
        if m - 1 >= 0:
            d[m - 1, m] = -1.0
    return d


def _mats():
    m1 = _dmat(128)
    m3 = np.zeros((128, 128), np.float16)
    m3[0:HB, 0:HB] = _dmat(HB)
    m3[HB:128, HB:128] = _dmat(HB)
    b1 = np.zeros((128, 128), np.float16)
    b1[0, 127] = 1.0      # T1 gy[127] += x[128] (= T3 p0)
    b2 = np.zeros((128, 128), np.float16)
    b2[HB, 127] = 1.0     # T2 gy[127] += X[128] (= T3 p64)
    b3 = np.zeros((128, 128), np.float16)
    b3[127, 0] = -1.0     # T3 gy[h128] -= x[127] (= T1 p127)
    b4 = np.zeros((128, 128), np.float16)
    b4[127, HB] = -1.0    # T3 gy[h128,tgt] -= X[127] (= T2 p127)
    i_ = np.eye(128, dtype=np.float16)
    return np.concatenate([m1, m3, b1, b2, b3, b4, i_], axis=1)


MATS_NP = _mats()


def _shard(vol, q):
    sh = np.zeros((S, H, WP), np.float16)
    d0 = DL * q - 1
    lo, hi = max(d0, 0), min(d0 + S, DVOL)
    sh[lo - d0 : hi - d0, :, 2 : 2 + W] = vol[lo:hi].astype(np.float16)
    return np.ascontiguousarray(sh.transpose(1, 0, 2)).reshape(H, FREE)


def make_in_maps(pred, target):
    pred = np.asarray(pred, dtype=np.float32).reshape(BATCH, DVOL, H, W)
    target = np.asarray(target, dtype=np.float32).reshape(BATCH, DVOL, H, W)
    maps = []
    for c in range(NCORES):
        b, q = divmod(c, NQ)
        maps.append(
            {
                "xp": _shard(pred[b], q),
                "xt": _shard(target[b], q),
                "mats": MATS_NP,
            }
        )
    return maps


def combine(results):
    spt = pt = 0.0
    for r in results:
        o = np.asarray(r["o"], dtype=np.float64)
        ns = o.shape[1] // 5
        spt += o[:, 0 : 3 * ns].sum()
        pt += o[:, 3 * ns : 4 * ns].sum() + o[0:HB, 4 * ns : 5 * ns].sum()
    dice = (2.0 * pt + EPS) / (spt + EPS)
    return np.float32(1.0 - dice)


def run_on_device(in_maps, **kwargs):
    from concourse.bass_utils import run_bass_kernel_spmd

    nc = get_nc()
    return run_bass_kernel_spmd(nc, in_maps, core_ids=list(range(NCORES)), **kwargs)


def kernel(pred, target):
    in_maps = make_in_maps(pred, target)
    res = run_on_device(in_maps)
    return combine(res.results)


if __name__ == "__main__":
    rng = np.random.default_rng(0)
    p = rng.random((2, 1, 192, 192, 192), np.float32)
    t = rng.random((2, 1, 192, 192, 192), np.float32)
    print(kernel(p, t))


# revision 6
# speedup vs baseline: 1.7538x; 1.0166x over previous
"""Kernel v3: 3-tile row packing — [pred h0:128], [tgt h0:128],
[pred h128:192 | tgt h128:192] — so every op uses all 128 partitions
(25% fewer columns than the 2-chunk layout) and all rows are valid.

Height-diff gy via per-tile shift matmuls; the 2-row cross-tile
boundaries (h=127/128 of each tensor) are stitched with extra matmuls
from the neighboring tile accumulated into the same PSUM. T3's
pb(pred)*tb(tgt) product needs lane alignment: DMA-realign rows 64:128
onto 0:64 then TSP. Dice needs only Σpb+Σtb so per-tile sqrt accums sum
host-side.
"""

import sys

sys.path.insert(0, "/opt/trn_rl_repo")

import numpy as np

BATCH = 2
DVOL = 192
H = 192
W = 192
NCORES = 8
NQ = 4
DL = DVOL // NQ      # 48
S = DL + 2           # 50
WP = W + 4           # 196
FREE = S * WP        # 9800
EPS = 1e-5
HB = 64              # T3 block height

_NC_CACHE = {}
_WAIT_CAP = 1


def _split_multiwait_json(bs: bytes) -> bytes:
    import json

    m = json.loads(bs)
    changed = False
    for fn in m.get("functions", []):
        for blk in fn.get("blocks", []):
            insts = blk.get("instructions")
            if not insts:
                continue
            out = []
            for ins in insts:
                si = ins.get("sync_info") or {}
                ow = si.get("on_wait") or []
                if len(ow) > _WAIT_CAP:
                    chunks = [
                        ow[i : i + _WAIT_CAP] for i in range(0, len(ow), _WAIT_CAP)
                    ]
                    for ci, ch in enumerate(chunks[:-1]):
                        out.append(
                            {
                                "debug": ins.get("debug", 0),
                                "engine": ins["engine"],
                                "ins": [],
                                "outs": [],
                                "is_reset_sema": False,
                                "name": f"{ins['name']}__w{ci}",
                                "opcode": "EventSemaphore",
                                "sync_info": {"on_update": [], "on_wait": ch},
                            }
                        )
                    si["on_wait"] = chunks[-1]
                    ins["sync_info"] = si
                    changed = True
                out.append(ins)
            blk["instructions"] = out
    if not changed:
        return bs
    return json.dumps(m).encode()


def _install_json_patch():
    import concourse.bass as bass

    if getattr(bass.Bass, "_bl_json_patched", False):
        return
    orig = bass.Bass.to_json_bytes

    def to_json_bytes(self, *a, **k):
        return _split_multiwait_json(orig(self, *a, **k))

    bass.Bass.to_json_bytes = to_json_bytes
    bass.Bass._bl_json_patched = True


def build_nc(
    repeats=1,
    sbsl=4,            # slices per sub-block
    vmode="psum_acc",  # "psum_acc" | "mixed"
    n_subpool=0,       # subs on Pool count — HW: gpsimd far slower than modeled, keep 0
    n_gx2act=6,        # fused squares on ACT (~17% of units, HW-swept)
    n_gz2act=0,
    lag=2,
    work_bufs=5,
    pb_bufs=3,
    dma_pieces=6,
):
    from contextlib import ExitStack

    import concourse.bass as bass
    import concourse.mybir as mybir
    from concourse import tile

    _install_json_patch()

    f16 = mybir.dt.float16
    f32 = mybir.dt.float32
    MULT = mybir.AluOpType.mult
    ADD = mybir.AluOpType.add
    SQUARE = mybir.ActivationFunctionType.Square
    SQRT = mybir.ActivationFunctionType.Sqrt
    AXX = mybir.AxisListType.X

    sbc = sbsl * WP
    OUTC = DL * WP
    nsb = OUTC // sbc
    assert nsb * sbc == OUTC
    MMW = []
    w0 = 0
    while w0 < sbc:
        MMW.append((w0, min(512, sbc - w0)))
        w0 += 512
    psum_banks = -(-sbc * 4 // 2048)   # per ps tile (unfused, fp32)
    if vmode == "mixed":
        psum_bufs = max(1, 8 // (2 * psum_banks))
    else:
        psum_bufs = min(4, max(2, 8 // psum_banks))

    nc = bass.Bass("TRN2", target_bir_lowering=False, debug=False)

    xp = nc.dram_tensor("xp", [H, FREE], f16, kind="ExternalInput")
    xt = nc.dram_tensor("xt", [H, FREE], f16, kind="ExternalInput")
    mats = nc.dram_tensor("mats", [128, 7 * 128], f16, kind="ExternalInput")
    nslots_ = DL * WP // (sbsl * WP)
    out = nc.dram_tensor("o", [128, 5 * nslots_], f32, kind="ExternalOutput")

    with tile.TileContext(nc) as tc, ExitStack() as ctx:
        const = ctx.enter_context(tc.tile_pool(name="const", bufs=1))
        xpool = ctx.enter_context(tc.tile_pool(name="x", bufs=1))
        work = ctx.enter_context(tc.tile_pool(name="work", bufs=work_bufs))
        pbp = ctx.enter_context(tc.tile_pool(name="pb", bufs=pb_bufs))
        accp = ctx.enter_context(tc.tile_pool(name="acc", bufs=1))
        psum = ctx.enter_context(
            tc.tile_pool(name="psum", bufs=psum_bufs, space="PSUM")
        )

        mt = const.tile([128, 7 * 128], f16, tag="mats")
        nc.sync.dma_start(mt[:], mats[:, :])
        M1 = mt[:, 0:128]          # within-tile shift for T1/T2
        M3 = mt[:, 128:256]        # block-diag shift for T3
        B1 = mt[:, 256:384]        # T3 -> T1 psum (+x[128] to row 127)
        B2 = mt[:, 384:512]        # T3 -> T2 psum
        B3 = mt[:, 512:640]        # T1 -> T3 psum (-x[127] to row 0)
        B4 = mt[:, 640:768]        # T2 -> T3 psum (-X[127] to row 64)
        I_ = mt[:, 768:896]        # identity
        eps_t = const.tile([128, 1], f32, tag="eps")
        nc.vector.memset(eps_t[:], EPS)

        # tiles: T1 = pred rows 0:128, T2 = tgt rows 0:128,
        #        T3 = pred rows 128:192 on p0:64, tgt rows 128:192 on p64:128
        T = {}
        for k in (1, 2, 3):
            T[k] = xpool.tile([128, FREE], f16, tag=f"t{k}", name=f"t{k}")
        first = sbc + 392
        csteps = [(0, first)]
        cstep = -(-(FREE - first) // dma_pieces)
        c = first
        while c < FREE:
            csteps.append((c, min(FREE, c + cstep)))
            c += cstep
        for c0_, c1_ in csteps:
            nc.sync.dma_start(T[1][:, c0_:c1_], xp[0:128, c0_:c1_])
            nc.sync.dma_start(T[2][:, c0_:c1_], xt[0:128, c0_:c1_])
            nc.sync.dma_start(T[3][0:HB, c0_:c1_], xp[128:192, c0_:c1_])
            nc.sync.dma_start(T[3][HB:128, c0_:c1_], xt[128:192, c0_:c1_])

        # accum slots
        SA = {}
        nslots = nsb
        for q in ("s1", "s2", "s3", "pa", "pb"):
            SA[q] = accp.tile([128, nslots], f32, tag=f"sa_{q}", name=f"sa_{q}")

        spsb = sbsl
        nu = 3 * nsb
        kpool = max(0, round(n_subpool * nu / 36))
        kgx = max(0, round(n_gx2act * nu / 36))
        kgz = max(0, round(n_gz2act * nu / 36))
        ucount = [0]
        units = []
        for rep in range(repeats):
            for sb in range(nsb):
                for tk in (3, 1, 2):
                    units.append((rep, tk, sb))

        # per-(tile,sb) shift matmul plans: (main_mat, [(bnd_mat, src_tile)...])
        PBTILE = {}  # (tk, sb) -> pb tile (for products)

        def emit_p1(unit):
            rep, tk, sb = unit
            u = ucount[0]
            ucount[0] += 1
            x = T[tk]
            c0 = sb * sbc
            g = work.tile([128, 2, sbc], f16, tag="g")
            on_pool = kpool > 0 and ((u % nu) * kpool) % nu < kpool
            eng = nc.gpsimd if on_pool else nc.vector
            eng.tensor_sub(
                g[:, 0, :], x[:, c0 + 392 : c0 + 392 + sbc], x[:, c0 : c0 + sbc]
            )
            eng2 = nc.gpsimd if on_pool else nc.vector
            eng2.tensor_sub(
                g[:, 1, :],
                x[:, c0 + 197 : c0 + 197 + sbc],
                x[:, c0 + 195 : c0 + 195 + sbc],
            )
            g2 = work.tile([128, 2, sbc], f16, tag="g2")
            if kgx > 0 and ((u % nu) * kgx) % nu < kgx:
                nc.scalar.activation(g2[:], g[:], SQUARE)
            else:
                nc.vector.tensor_mul(g2[:], g[:], g[:])
            gx2 = g2[:, 0, :]
            gz2 = g2[:, 1, :]
            # gy into psum: main shift matmul + boundary stitches
            ps = psum.tile([128, sbc], f32, tag="ps")
            if tk == 1:
                plan = [(M1, T[1], True, False), (B1, T[3], False, True)]
            elif tk == 2:
                plan = [(M1, T[2], True, False), (B2, T[3], False, True)]
            else:
                plan = [
                    (M3, T[3], True, False),
                    (B3, T[1], False, False),
                    (B4, T[2], False, True),
                ]
            for w0, wn in MMW:
                for mat, src, st, sp in plan:
                    nc.tensor.matmul(
                        ps[:, w0 : w0 + wn],
                        mat,
                        src[:, 196 + c0 + w0 : 196 + c0 + w0 + wn],
                        start=st,
                        stop=sp,
                    )
            return (u, unit, c0, gx2, gz2, ps)

        def emit_p2(st_):
            u, unit, c0, gx2, gz2, ps = st_
            rep, tk, sb = unit
            if vmode == "psum_acc":
                q_ = work.tile([128, sbc], f16, tag="q")
                nc.scalar.activation(q_[:], ps[:], SQUARE)
                for w0, wn in MMW:
                    nc.tensor.matmul(
                        ps[:, w0 : w0 + wn], I_, gx2[:, w0 : w0 + wn],
                        start=True, stop=False,
                    )
                    nc.tensor.matmul(
                        ps[:, w0 : w0 + wn], I_, gz2[:, w0 : w0 + wn],
                        start=False, stop=False,
                    )
                    nc.tensor.matmul(
                        ps[:, w0 : w0 + wn], I_, q_[:, w0 : w0 + wn],
                        start=False, stop=True,
                    )
                vsrc = ps
            else:  # mixed: ACT squares psum->psum, identities accumulate
                ps2 = psum.tile([128, sbc], f32, tag="ps2")
                nc.scalar.activation(ps2[:], ps[:], SQUARE)
                for w0, wn in MMW:
                    nc.tensor.matmul(
                        ps2[:, w0 : w0 + wn], I_, gx2[:, w0 : w0 + wn],
                        start=False, stop=False,
                    )
                    nc.tensor.matmul(
                        ps2[:, w0 : w0 + wn], I_, gz2[:, w0 : w0 + wn],
                        start=False, stop=True,
                    )
                vsrc = ps2
            # sqrt(v + eps) with fused row-sum accum
            pb = pbp.tile([128, spsb * W], f16, tag=f"pb{tk}")
            v3 = vsrc[:].rearrange("p (s w) -> p s w", s=spsb)
            pb3 = pb[:].rearrange("p (s w) -> p s w", s=spsb)
            slot = sb
            nc.scalar.activation(
                pb3[:, :, :],
                v3[:, :, 2 : 2 + W],
                SQRT,
                bias=eps_t[:],
                accum_out=SA[f"s{tk}"][:, slot : slot + 1],
            )
            PBTILE[(tk, sb)] = pb
            # products
            if tk == 2:
                prod = work.tile([128, spsb * W], f16, tag="prod")
                nc.vector.scalar_tensor_tensor(
                    prod[:, :],
                    PBTILE[(1, sb)][:, :],
                    1.0,
                    pb[:, :],
                    op0=MULT,
                    op1=MULT,
                    accum_out=SA["pa"][:, slot : slot + 1],
                )
            elif tk == 3:
                pbr = work.tile([HB, spsb * W], f16, tag="pbr")
                nc.sync.dma_start(pbr[:, :], pb[HB:128, :])
                prod3 = work.tile([HB, spsb * W], f16, tag="prod3")
                nc.vector.scalar_tensor_tensor(
                    prod3[:, :],
                    pb[0:HB, :],
                    1.0,
                    pbr[:, :],
                    op0=MULT,
                    op1=MULT,
                    accum_out=SA["pb"][0:HB, slot : slot + 1],
                )

        pend = []
        for unit in units:
            pend.append(emit_p1(unit))
            if len(pend) > lag:
                emit_p2(pend.pop(0))
        for st_ in pend:
            emit_p2(st_)

        # ship raw accum slots; host reduces (saves tail reduces+DMAs)
        for col, q in enumerate(("s1", "s2", "s3", "pa", "pb")):
            pc = HB if q == "pb" else 128
            nc.sync.dma_start(
                out[0:pc, col * nslots : (col + 1) * nslots],
                SA[q][0:pc, 0:nslots],
            )

    return nc


def get_nc():
    if "nc" not in _NC_CACHE:
        _NC_CACHE["nc"] = build_nc()
    return _NC_CACHE["nc"]


def _dmat(k):
    d = np.zeros((k, k), np.float16)
    for m in range(k):
        if m + 1 < k:
            d[m + 1, m] = 1.0
        if m - 1 >= 0:
            d[m - 1, m] = -1.0
    return d


def _mats():
    m1 = _dmat(128)
    m3 = np.zeros((128, 128), np.float16)
    m3[0:HB, 0:HB] = _dmat(HB)
    m3[HB:128, HB:128] = _dmat(HB)
    b1 = np.zeros((128, 128), np.float16)
    b1[0, 127] = 1.0      # T1 gy[127] += x[128] (= T3 p0)
    b2 = np.zeros((128, 128), np.float16)
    b2[HB, 127] = 1.0     # T2 gy[127] += X[128] (= T3 p64)
    b3 = np.zeros((128, 128), np.float16)
    b3[127, 0] = -1.0     # T3 gy[h128] -= x[127] (= T1 p127)
    b4 = np.zeros((128, 128), np.float16)
    b4[127, HB] = -1.0    # T3 gy[h128,tgt] -= X[127] (= T2 p127)
    i_ = np.eye(128, dtype=np.float16)
    return np.concatenate([m1, m3, b1, b2, b3, b4, i_], axis=1)


MATS_NP = _mats()


def _shard(vol, q):
    sh = np.zeros((S, H, WP), np.float16)
    d0 = DL * q - 1
    lo, hi = max(d0, 0), min(d0 + S, DVOL)
    sh[lo - d0 : hi - d0, :, 2 : 2 + W] = vol[lo:hi].astype(np.float16)
    return np.ascontiguousarray(sh.transpose(1, 0, 2)).reshape(H, FREE)


def make_in_maps(pred, target):
    pred = np.asarray(pred, dtype=np.float32).reshape(BATCH, DVOL, H, W)
    target = np.asarray(target, dtype=np.float32).reshape(BATCH, DVOL, H, W)
    maps = []
    for c in range(NCORES):
        b, q = divmod(c, NQ)
        maps.append(
            {
                "xp": _shard(pred[b], q),
                "xt": _shard(target[b], q),
                "mats": MATS_NP,
            }
        )
    return maps


def combine(results):
    spt = pt = 0.0
    for r in results:
        o = np.asarray(r["o"], dtype=np.float64)
        ns = o.shape[1] // 5
        spt += o[:, 0 : 3 * ns].sum()
        pt += o[:, 3 * ns : 4 * ns].sum() + o[0:HB, 4 * ns : 5 * ns].sum()
    dice = (2.0 * pt + EPS) / (spt + EPS)
    return np.float32(1.0 - dice)


def run_on_device(in_maps, **kwargs):
    from concourse.bass_utils import run_bass_kernel_spmd

    nc = get_nc()
    return run_bass_kernel_spmd(nc, in_maps, core_ids=list(range(NCORES)), **kwargs)


def kernel(pred, target):
    in_maps = make_in_maps(pred, target)
    res = run_on_device(in_maps)
    return combine(res.results)


if __name__ == "__main__":
    rng = np.random.default_rng(0)
    p = rng.random((2, 1, 192, 192, 192), np.float32)
    t = rng.random((2, 1, 192, 192, 192), np.float32)
    print(kernel(p, t))
